# revision 4
# baseline (speedup 1.0000x reference)
"""Trainium2 Bass kernel for nn_BaselineGNN (GNN message passing).

Strategy (8 NeuronCores, SPMD):
  - Node-partition the graph: core c owns dst nodes [c*12500, (c+1)*12500).
  - Edges live on the core that owns their dst, grouped by 128-node dst block
    and by src HALF (the m table split in 2; idx are int16 BIASED by -25088 so
    one gather call addresses a 65536-row window -- the SWDGE ucode
    sign-extends idx via IVP_MULUSAN_2X32).  Self-loops are a single identity
    matmul per block from the SBUF-resident m tile.
  - Gather calls are LARGE (one per (block-group, half)); SWDGE descriptor
    generation costs ~1us fixed + 0.34ns/row, so few big calls beat many
    small ones by ~8x.
  - Phase 0: global-context encoder u' = relu(gf@Wg+bg)@Wc2+bc (tiny, fp32,
    computed redundantly on every core) -> u_buf DRAM.
  - Phase 1 (own slice, per 7-block group): u' expanded per node with one
    dma_gather per group; h0 = relu(x@Wc1 + u'[batch]) kept fp16 as h0T;
    m = relu(h0@Wm+bm) kept fp16 (SBUF + DRAM m_slice).
  - AllGather m (fp16) so every core can gather any src row.
  - Phase 2: per dst block, aggr^T accumulates in PSUM as
      m_block^T (self loops, via identity matmul)
      + sum_{chunk} m_gathered[128e,128h]^T @ onehot(dst)[128e,128d]
      + Ws^T @ h0^T
    then h^T = relu(aggr^T + bs), out^T = Wo^T@h^T + bo.
  - All idx pads are 0 (a valid row), so every call gathers full capacity:
    no count registers, no trailing -1 trim hazards, and mg tiles always
    hold finite data (0-coefficient one-hot columns ignore pad rows).
"""
import contextlib
import ctypes
import os
import sys

sys.path.insert(0, "/opt/trn_rl_repo")

import numpy as np

import concourse.bass as bass
import concourse.bacc as bacc
import concourse.tile as tile
from concourse import mybir
from concourse.library_config import mlp
from concourse.masks import make_identity

N_NODES, N_EDGES, N_GRAPHS = 100000, 1600000, 1024
IN_LOCAL, IN_GLOBAL, HIDDEN, NUM_CLASSES = 16, 8, 128, 2
P = 128
N_CORES = 8
SLICE = N_NODES // N_CORES            # 12500
NBLK = -(-SLICE // P)                 # 98
PAD_SLICE = NBLK * P                  # 12544
GBLK = N_GRAPHS // P                  # 8
NHALF = 2
HALF_ROWS = N_CORES * PAD_SLICE // NHALF   # 50176
IDX_BIAS = HALF_ROWS // 2                  # 25088 (int16 window center)
G = 7                                  # blocks per group
NGRP = NBLK // G                       # 14

f32 = mybir.dt.float32
f16 = mybir.dt.float16
i16 = mybir.dt.int16

_prog_cache: dict = {}
last_run: dict = {}


# --------------------------------------------------------------------------
# device program
# --------------------------------------------------------------------------
def _build(kbh, kcall):
    """kbh[b][h]: chunk capacity per (block, half) (max over cores, shared
    static schedule).  kcall[g][h] = sum_b kbh[b][h] for b in group g."""
    kbh_max = int(max(max(r) for r in kbh))
    kcap = int(max(max(r) for r in kcall))
    ctot = int(sum(sum(r) for r in kbh))         # total chunk columns
    ixe_cols = sum(kcall[g][h] * P // 16 for g in range(NGRP) for h in range(NHALF))

    nc = bacc.Bacc("TRN2", target_bir_lowering=False, debug=False,
                   num_devices=N_CORES, num_swdge_queues=4)

    def inp(name, shape, dt):
        return nc.dram_tensor(name, shape, dt, kind="ExternalInput").ap()

    xT_d = inp("xT", [IN_LOCAL, PAD_SLICE], f32)
    gfT_d = inp("gfT", [IN_GLOBAL, N_GRAPHS], f32)
    ixu_d = inp("ixu", [P, NGRP * (G * P // 16)], i16)
    ixe_d = inp("ixe", [P, ixe_cols], i16)
    dstT_d = inp("dstT", [P, ctot], f16)
    iota_d = inp("iota", [P, kbh_max * P], f16)
    Wg_d = inp("Wg", [IN_GLOBAL, HIDDEN], f32)
    Wc1_d = inp("Wc1", [IN_LOCAL, HIDDEN], f32)
    Wc2_d = inp("Wc2", [HIDDEN, HIDDEN], f32)
    Wm_d = inp("Wm", [HIDDEN, HIDDEN], f16)
    Ws_d = inp("Ws", [HIDDEN, HIDDEN], f16)
    Wo_d = inp("Wo", [HIDDEN, NUM_CLASSES], f16)
    bg_d = inp("bg_c", [HIDDEN, 1], f32)
    bc_d = inp("bc_b", [P, HIDDEN], f32)
    bm_d = inp("bm_b", [P, HIDDEN], f32)
    bs_d = inp("bs_c", [HIDDEN, 1], f32)
    bo_d = inp("bo_c", [NUM_CLASSES, 1], f32)
    id16_d = inp("id16", [P, P], f16)
    outT_d = nc.dram_tensor("outT", [NUM_CLASSES, SLICE], f32,
                            kind="ExternalOutput").ap()

    u_buf = nc.dram_tensor("u_buf", [N_GRAPHS, HIDDEN], f32).ap()
    m_slice = nc.dram_tensor("m_slice", [PAD_SLICE, HIDDEN], f16).ap()
    m_full = nc.dram_tensor("m_full", [N_CORES * PAD_SLICE, HIDDEN], f16,
                            addr_space="Shared").ap()

    AF = mybir.ActivationFunctionType
    OP = mybir.AluOpType

    with tile.TileContext(nc) as tc:
        with (
            tc.tile_pool(name="const", bufs=1) as cpool,
            tc.tile_pool(name="persist", bufs=1) as ppool,
            tc.tile_pool(name="work", bufs=3) as wpool,
            tc.tile_pool(name="sbig", bufs=4) as spool,
            tc.tile_pool(name="uexp", bufs=2) as upool,
            tc.tile_pool(name="xg", bufs=2) as xgpool,
            tc.tile_pool(name="ixe", bufs=3) as ixpool,
            tc.tile_pool(name="mg", bufs=4) as mgpool,
            tc.tile_pool(name="og", bufs=2) as ogpool,
            tc.tile_pool(name="ps_a", bufs=2, space="PSUM") as ps_a,
            tc.tile_pool(name="ps_b", bufs=3, space="PSUM") as ps_b,
            tc.tile_pool(name="ps_t", bufs=2, space="PSUM") as ps_t,
            tc.tile_pool(name="ps_o", bufs=1, space="PSUM") as ps_o,
        ):
            nc.gpsimd.load_library(mlp)

            def ctile(name, ap, shape, dt):
                t = cpool.tile(shape, dt, tag=f"c_{name}")
                nc.sync.dma_start(t[:], ap[:])
                return t

            Wg_t = ctile("Wg", Wg_d, [IN_GLOBAL, HIDDEN], f32)
            Wc1_t = ctile("Wc1", Wc1_d, [IN_LOCAL, HIDDEN], f32)
            Wc2_t = ctile("Wc2", Wc2_d, [HIDDEN, HIDDEN], f32)
            Wm_t = ctile("Wm", Wm_d, [HIDDEN, HIDDEN], f16)
            Ws_t = ctile("Ws", Ws_d, [HIDDEN, HIDDEN], f16)
            Wo_t = ctile("Wo", Wo_d, [HIDDEN, NUM_CLASSES], f16)
            bg_t = ctile("bg", bg_d, [HIDDEN, 1], f32)
            bc_t = ctile("bc", bc_d, [P, HIDDEN], f32)
            bm_t = ctile("bm", bm_d, [P, HIDDEN], f32)
            bs_t = ctile("bs", bs_d, [HIDDEN, 1], f32)
            bo_t = ctile("bo", bo_d, [NUM_CLASSES, 1], f32)
            gfT_t = ctile("gfT", gfT_d, [IN_GLOBAL, N_GRAPHS], f32)
            id16_t = ctile("id16", id16_d, [P, P], f16)
            iota_t = ctile("iota", iota_d, [P, kbh_max * P], f16)

            ident = cpool.tile([P, P], f32)
            make_identity(nc, ident[:])

            ixu_t = ppool.tile([P, NGRP * (G * P // 16)], i16)
            nc.sync.dma_start(ixu_t[:], ixu_d[:])
            dstT_t = ppool.tile([P, ctot], f16)
            nc.sync.dma_start(dstT_t[:], dstT_d[:])

            h0T_t = ppool.tile([HIDDEN, PAD_SLICE], f16)    # 3.2 MB persistent
            m16_t = ppool.tile([P, PAD_SLICE], f16)         # 3.2 MB persistent

            # ---------------- phase 0: global encoder ----------------
            for g in range(GBLK):
                gsl = slice(g * P, (g + 1) * P)
                ps1 = ps_b.tile([P, P], f32, tag="pb")
                nc.tensor.matmul(out=ps1[:], lhsT=Wg_t[:], rhs=gfT_t[:, gsl],
                                 start=True, stop=True)
                rT = wpool.tile([P, P], f32, tag="rT")
                nc.scalar.activation(out=rT[:], in_=ps1[:], func=AF.Relu,
                                     bias=bg_t[:, :1])
                ps2 = ps_b.tile([P, P], f32, tag="pb")
                nc.tensor.matmul(out=ps2[:], lhsT=Wc2_t[:], rhs=rT[:],
                                 start=True, stop=True)
                uT = wpool.tile([P, P], f32, tag="uT")
                nc.vector.tensor_copy(out=uT[:], in_=ps2[:])
                ps3 = ps_t.tile([P, P], f32, tag="pt")
                nc.tensor.transpose(out=ps3[:], in_=uT[:], identity=ident[:])
                ub = wpool.tile([P, P], f32, tag="ublk")
                nc.vector.tensor_tensor(out=ub[:], in0=ps3[:], in1=bc_t[:],
                                        op=OP.add)
                nc.sync.dma_start(u_buf[gsl, :], ub[:])

            # ---------------- phase 1: h0 / m on own slice ----------------
            ucols = G * P // 16
            for g in range(NGRP):
                gsl = slice(g * G * P, (g + 1) * G * P)
                xg = xgpool.tile([IN_LOCAL, G * P], f32, tag="xg")
                nc.sync.dma_start(xg[:], xT_d[:, gsl])
                uexp = upool.tile([P, G, HIDDEN], f32, tag="uexp")
                nc.gpsimd.dma_gather(
                    uexp[:], u_buf[:], ixu_t[:, g * ucols:(g + 1) * ucols],
                    G * P, G * P, HIDDEN, single_packet=False,
                    queue_num=g % 4)
                for j in range(G):
                    b = g * G + j
                    bsl = slice(b * P, (b + 1) * P)
                    psh = ps_b.tile([P, P], f32, tag="pb")
                    nc.tensor.matmul(out=psh[:], lhsT=Wc1_t[:],
                                     rhs=xg[:, j * P:(j + 1) * P],
                                     start=True, stop=False)
                    nc.tensor.matmul(out=psh[:], lhsT=uexp[:, j, :],
                                     rhs=ident[:], is_transpose=True,
                                     start=False, stop=True)
                    nc.vector.tensor_scalar_max(out=h0T_t[:, bsl],
                                                in0=psh[:], scalar1=0.0)
                    psm = ps_b.tile([P, P], f32, tag="pb")
                    nc.tensor.matmul(out=psm[:], lhsT=h0T_t[:, bsl], rhs=Wm_t[:],
                                     start=True, stop=True)
                    nc.vector.tensor_tensor(out=m16_t[:, bsl], in0=psm[:],
                                            in1=bm_t[:], op=OP.add)
                    nc.vector.tensor_scalar_max(out=m16_t[:, bsl],
                                                in0=m16_t[:, bsl], scalar1=0.0)
                # one m_slice DMA per group: [128, G*128] tile -> G*128 rows
                nc.sync.dma_start(
                    m_slice[gsl, :].rearrange("(b p) h -> p b h", p=P),
                    m16_t[:, gsl].rearrange("p (b h) -> p b h", h=HIDDEN))

            # ---------------- allgather m ----------------
            nc.gpsimd.collective_compute(
                "AllGather", OP.bypass,
                replica_groups=[list(range(N_CORES))],
                ins=[m_slice[:]], outs=[m_full[:]])

            # ---------------- phase 2: scatter-add + update + readout ------
            iota_v = iota_t[:].rearrange("p (k f) -> p k f", k=kbh_max)
            # chunk-column offset per (g, h, b): in (g, h, blocks...) order
            coff = {}
            ixoff = {}
            c_acc = 0
            ix_acc = 0
            for g in range(NGRP):
                for h in range(NHALF):
                    ixoff[(g, h)] = ix_acc
                    ix_acc += kcall[g][h] * P // 16
                    for j in range(G):
                        b = g * G + j
                        coff[(g, h, b)] = c_acc
                        c_acc += kbh[b][h]

            for g in range(NGRP):
                mg = {}
                for h in range(NHALF):
                    K = kcall[g][h]
                    if K == 0:
                        continue
                    ixw = K * P // 16
                    ixt = ixpool.tile([P, (kcap * P) // 16], i16, tag="ixe")
                    nc.sync.dma_start(
                        ixt[:, :ixw],
                        ixe_d[:, ixoff[(g, h)]:ixoff[(g, h)] + ixw])
                    mgt = mgpool.tile([P, kcap, HIDDEN], f16, tag="mg")
                    base = h * HALF_ROWS + IDX_BIAS
                    nc.gpsimd.dma_gather(
                        mgt[:, :K, :], m_full[base:base + HALF_ROWS - IDX_BIAS, :],
                        ixt[:, :ixw], K * P, K * P, HIDDEN,
                        single_packet=False, queue_num=(2 * g + h) % 4)
                    mg[h] = mgt

                og = ogpool.tile([NUM_CLASSES, G * P], f32, tag="og")
                for j in range(G):
                    b = g * G + j
                    bsl = slice(b * P, (b + 1) * P)
                    pa = ps_a.tile([HIDDEN, P], f32, tag="pa")
                    # self loops: aggrT += m_block^T
                    nc.tensor.matmul(out=pa[:], lhsT=m16_t[:, bsl],
                                     rhs=id16_t[:], start=True, stop=False)
                    for h in range(NHALF):
                        k = kbh[b][h]
                        if k == 0:
                            continue
                        c0 = coff[(g, h, b)]
                        S = spool.tile([P, kbh_max, P], f16, tag="S")
                        nc.vector.tensor_tensor(
                            out=S[:, :k, :],
                            in0=dstT_t[:, c0:c0 + k].to_broadcast([P, k, P]),
                            in1=iota_v[:, :k, :], op=OP.is_equal)
                        off = c0 - coff[(g, h, g * G)]  # chunk offset in call
                        for kk in range(k):
                            nc.tensor.matmul(out=pa[:],
                                             lhsT=mg[h][:, off + kk, :],
                                             rhs=S[:, kk, :],
                                             start=False, stop=False)
                    nc.tensor.matmul(out=pa[:], lhsT=Ws_t[:], rhs=h0T_t[:, bsl],
                                     start=False, stop=True)
                    hT = wpool.tile([HIDDEN, P], f16, tag="hT")
                    nc.scalar.activation(out=hT[:], in_=pa[:], func=AF.Relu,
                                         bias=bs_t[:, :1])
                    po = ps_o.tile([NUM_CLASSES, P], f32, tag="po")
                    nc.tensor.matmul(out=po[:], lhsT=Wo_t[:], rhs=hT[:],
                                     start=True, stop=True)
                    nc.scalar.activation(out=og[:, j * P:(j + 1) * P],
                                         in_=po[:], func=AF.Identity,
                                         bias=bo_t[:, :1])
                w = min(SLICE, (g + 1) * G * P) - g * G * P
                nc.sync.dma_start(outT_d[:, g * G * P:g * G * P + w],
                                  og[:, :w])

    nc.compile()
    return nc


# --------------------------------------------------------------------------
# host side
# --------------------------------------------------------------------------
def _wrap16(ix):
    """dma_gather int16 index layout: [16, n/16] wrapped, tiled to 128 parts."""
    return np.tile(ix.reshape(-1, 16).T, (8, 1))


def _preprocess(inputs):
    x = np.asarray(inputs["x"], dtype=np.float32)
    ei = np.asarray(inputs["edge_index"]).astype(np.int64)
    batch = np.asarray(inputs["batch"]).astype(np.int64)
    gf = np.asarray(inputs["global_feat"], dtype=np.float32)
    W = {k: np.ascontiguousarray(np.asarray(inputs[k], dtype=np.float32))
         for k in ("Wg", "bg", "Wc", "bc", "Wm", "bm", "Ws", "bs", "Wo", "bo")}

    src_all, dst_all = ei[0], ei[1]
    src_row = (src_all // SLICE) * PAD_SLICE + (src_all % SLICE)
    half_all = src_row // HALF_ROWS
    idx16_all = (src_row % HALF_ROWS - IDX_BIAS).astype(np.int16)
    core_of = dst_all // SLICE

    # per (core, block, half) counts and sorted edge arrays
    per_core = []
    counts = np.zeros((N_CORES, NBLK, NHALF), np.int64)
    for c in range(N_CORES):
        sel = np.nonzero(core_of == c)[0]
        d_loc = dst_all[sel] - c * SLICE
        blk = d_loc // P
        h = half_all[sel]
        key = blk * NHALF + h
        # sort by (block, half, idx) -- ascending idx makes the last real
        # idx of each (b,h) the max (never negative in practice)
        order = np.lexsort((idx16_all[sel], key))
        sel = sel[order]
        cnt = np.bincount(key[order], minlength=NBLK * NHALF
                          ).reshape(NBLK, NHALF)
        counts[c] = cnt
        per_core.append((sel, (d_loc[order] % P).astype(np.float16),
                         idx16_all[sel], cnt))

    kbh = -(-counts.max(axis=0) // P)             # [NBLK, NHALF] chunk caps
    kcall = np.zeros((NGRP, NHALF), np.int64)
    for g in range(NGRP):
        kcall[g] = kbh[g * G:(g + 1) * G].sum(axis=0)
    kbh_max = int(kbh.max())
    ctot = int(kbh.sum())

    iota_np = np.tile(np.arange(P, dtype=np.float16), (P, kbh_max))
    shared = {
        "gfT": np.ascontiguousarray(gf.T),
        "iota": iota_np,
        "id16": np.eye(P, dtype=np.float16),
        "Wg": W["Wg"],
        "Wc1": np.ascontiguousarray(W["Wc"][:IN_LOCAL]),
        "Wc2": np.ascontiguousarray(W["Wc"][IN_LOCAL:]),
        "Wm": W["Wm"].astype(np.float16),
        "Ws": W["Ws"].astype(np.float16),
        "Wo": W["Wo"].astype(np.float16),
        "bg_c": W["bg"].reshape(HIDDEN, 1),
        "bc_b": np.tile(W["bc"], (P, 1)),
        "bm_b": np.tile(W["bm"], (P, 1)),
        "bs_c": W["bs"].reshape(HIDDEN, 1),
        "bo_c": W["bo"].reshape(NUM_CLASSES, 1),
    }

    in_maps = []
    for c in range(N_CORES):
        sel, d128, l16, cnt = per_core[c]
        # chunk-column layout: for g: for h: for b in group: kbh[b][h] chunks
        dstT = np.full((ctot, P), -1.0, np.float16)
        ixe_parts = []
        # per-(b,h) start offsets into the sorted edge array
        flat_cnt = cnt.reshape(-1)                 # [(b,h)]
        starts = np.cumsum(flat_cnt) - flat_cnt
        c_acc = 0
        for g in range(NGRP):
            for h in range(NHALF):
                K = int(kcall[g][h])
                if K == 0:
                    continue
                ix_pad = np.zeros(K * P, np.int16)
                pos = 0
                for j in range(G):
                    b = g * G + j
                    n = int(cnt[b][h])
                    s0 = int(starts[b * NHALF + h])
                    k = int(kbh[b][h])
                    ix_pad[pos:pos + n] = l16[s0:s0 + n]
                    # dst one-hot cols: chunk c_acc..c_acc+k-1
                    dd = dstT[c_acc:c_acc + k].reshape(-1)
                    dd[:n] = d128[s0:s0 + n]
                    c_acc += k
                    pos += k * P
                ixe_parts.append(_wrap16(ix_pad))
        ixe = np.concatenate(ixe_parts, axis=1)
        dstT_in = np.ascontiguousarray(dstT.T)     # [P, ctot]

        bpad = np.zeros(PAD_SLICE, np.int16)
        bpad[:SLICE] = batch[c * SLICE:(c + 1) * SLICE]
        ixu = np.concatenate(
            [_wrap16(bpad[g * G * P:(g + 1) * G * P]) for g in range(NGRP)],
            axis=1)

        xT = np.zeros((IN_LOCAL, PAD_SLICE), np.float32)
        xT[:, :SLICE] = x[c * SLICE:(c + 1) * SLICE].T

        m = dict(shared)
        m.update({"xT": xT, "ixu": ixu, "ixe": ixe, "dstT": dstT_in})
        in_maps.append(m)

    kbh_t = tuple(tuple(int(v) for v in r) for r in kbh)
    kcall_t = tuple(tuple(int(v) for v in r) for r in kcall)
    return kbh_t, kcall_t, in_maps


# --------------------------------------------------------------------------
# profiling hook (NTFF via the axon PJRT .so; absent module in this image)
# --------------------------------------------------------------------------
def _profile_hook():
    so = "/opt/axon/libaxon_pjrt.so"
    if not os.path.exists(so):
        return None
    lib = ctypes.CDLL(so)
    if not hasattr(lib, "axon_start_nrt_profile"):
        return None
    lib.axon_start_nrt_profile.argtypes = [ctypes.POINTER(ctypes.c_int64),
                                           ctypes.c_size_t]
    lib.axon_start_nrt_profile.restype = ctypes.c_int64
    lib.axon_stop_nrt_profile.argtypes = [ctypes.c_char_p]
    lib.axon_stop_nrt_profile.restype = ctypes.c_int64

    @contextlib.contextmanager
    def hook(output_dir, device_ids):
        import jax
        jax.devices()
        if device_ids:
            ids = (ctypes.c_int64 * len(device_ids))(*device_ids)
            rc = lib.axon_start_nrt_profile(ids, len(device_ids))
        else:
            rc = lib.axon_start_nrt_profile(None, 0)
        if rc != 0:
            raise RuntimeError(f"axon_start_nrt_profile rc={rc}")
        try:
            yield
        finally:
            n = lib.axon_stop_nrt_profile(str(output_dir).encode())
            print(f"profile: {n} file(s) written to {output_dir}",
                  file=sys.stderr)

    return hook


def _run(nc, in_maps):
    from concourse import bass2jax
    trace_dir = os.environ.get("GNN_TRACE_DIR", "")
    if not trace_dir:
        return bass2jax.run_bass_via_pjrt(nc, in_maps, n_cores=N_CORES)
    hook = _profile_hook()
    if hook is None:
        return bass2jax.run_bass_via_pjrt(nc, in_maps, n_cores=N_CORES)
    import time as _time
    trace_dir = os.path.join(trace_dir, f"run_{int(_time.time()*1000)}")
    os.makedirs(trace_dir, exist_ok=True)
    last_run["trace_dir"] = trace_dir
    trace_cores = [int(t) for t in
                   os.environ.get("GNN_TRACE_CORES", "0").split(",")]
    with hook(trace_dir, trace_cores):
        results = bass2jax.run_bass_via_pjrt(nc, in_maps, n_cores=N_CORES)
    try:
        from concourse._compat import FishPath
        import gauge.profiler as gprof
        profile = gprof.Profile(
            profile_path=FishPath(trace_dir), kernel_dev_mode=True,
            profile_on_exit=False, bass_kernel=nc.m,
            offline_processing=True, fname="*_body*")
        profile.convert_ntffs_to_json(tuple(trace_cores))
        j = profile.load_json(trace_cores[0])
        last_run["summary"] = j["summary"][0] if j else None
        last_run["exec_time_ns"] = (
            int(j["summary"][0]["total_time"] * 1e9) if j else None)
        last_run["profile_json"] = str(profile.json_path(trace_cores[0]))
    except Exception as e:  # profiling must never break the run
        print(f"profile post-processing failed: {e}", file=sys.stderr)
    return results


def kernel(**inputs) -> np.ndarray:
    kbh, kcall, in_maps = _preprocess(inputs)
    key = (kbh, kcall)
    nc = _prog_cache.get(key)
    if nc is None:
        nc = _build(kbh, kcall)
        _prog_cache[key] = nc
    last_run.clear()
    results = _run(nc, in_maps)
    outT = np.concatenate([results[c]["outT"] for c in range(N_CORES)], axis=1)
    return np.ascontiguousarray(outT.T.astype(np.float32))


# revision 13
# speedup vs baseline: 1.3744x; 1.3744x over previous
"""Trainium2 Bass kernel for nn_BaselineGNN (GNN message passing).

Strategy (8 NeuronCores, SPMD):
  - Node-partition the graph: core c owns dst nodes [c*12500, (c+1)*12500).
  - m table rows are HALF-major: m_full_h row = core*6272 + (node - h*6272)
    for node half h.  Gather idx are int16 BIASED by -25088 so one call
    addresses a whole 50176-row half (the SWDGE ucode sign-extends idx via
    IVP_MULUSAN_2X32; the in_ap base points at row 25088 of the half).
  - One dma_gather per (dst block, half): SWDGE descriptor generation costs
    ~4ns/row with ~zero fixed cost, so the lever is processing exactly the
    real rows.  Edge counts are host-EQUALIZED across cores per (b,h) (pad
    edges: idx 0, dst -1) so num_idxs is a shared static constant -- no
    count registers.  Within (b,h) idx are sorted ascending and pads are 0,
    so the ucode's trailing-negative-idx trim never eats real edges.
  - u expansion (u'[batch]) via per-core ROTATED graph space: core c gets
    gf columns (base_c + j) % 1024 where base_c = batch[c*12500], so its
    graphs live at rotated ids [0, ~140) and two 128-graph windows cover
    every core.  Phase 0 computes u_rot [128, 2, H]; phase 1 adds
    u'[batch] into h0 with two one-hot matmuls per block (B is input data).
  - AllGather is split into two half AGs issued from the SCALAR engine (so
    the collective wait never blocks GpSimd).  Phase 2 is software-
    pipelined with depth D: block b's half-0 chunks accumulate while AG1
    lands; half-1 chunks close the PSUM tile D blocks later.
  - Per dst block, aggr^T accumulates in PSUM as
      m_block^T (self loops, via identity matmul)
      + sum_{half,chunk} m_gathered[128e,128h]^T @ onehot(dst)[128e,128d]
      + Ws^T @ h0^T   (all fp16 matmuls)
    then h^T = relu(aggr^T + bs), out^T = Wo^T@h^T + bo.
  - 10 dedicated warm-up gathers (from id16, idx 0) run before phase 0
    while GpSimd is idle, filling every mg pool slot with finite fp16 so
    0-coefficient one-hot columns never hit NaN lanes.
"""
import contextlib
import ctypes
import os
import sys

sys.path.insert(0, "/opt/trn_rl_repo")

import numpy as np

import concourse.bass as bass
import concourse.bacc as bacc
import concourse.tile as tile
from concourse import mybir
from concourse.library_config import mlp
from concourse.masks import make_identity

N_NODES, N_EDGES, N_GRAPHS = 100000, 1600000, 1024
IN_LOCAL, IN_GLOBAL, HIDDEN, NUM_CLASSES = 16, 8, 128, 2
P = 128
N_CORES = 8
SLICE = N_NODES // N_CORES            # 12500
NBLK = -(-SLICE // P)                 # 98
PAD_SLICE = NBLK * P                  # 12544
NHALF = 2
HBLK = NBLK // NHALF                  # 49 blocks per half
HNODES = HBLK * P                     # 6272 local nodes per half
HALF_ROWS = N_CORES * HNODES          # 50176 rows per half table
IDX_BIAS = HALF_ROWS // 2             # 25088
NWIN = 2                              # rotated-graph windows per core
G = 7                                 # blocks per out-write group
D = 4                                 # phase-2 software pipeline depth
MG_BUFS = 10

f32 = mybir.dt.float32
f16 = mybir.dt.float16
i16 = mybir.dt.int16

_prog_cache: dict = {}
last_run: dict = {}


# --------------------------------------------------------------------------
# device program
# --------------------------------------------------------------------------
def _build(kbh, nbh):
    """kbh[b][h]: chunk capacity per (block, half); nbh[b][h]: equalized
    edge count (max over cores, <= kbh*128).  Shared across cores."""
    kbh_max = int(max(max(r) for r in kbh))
    ctot = int(sum(sum(r) for r in kbh))         # total chunk columns
    ixe_cols = ctot * P // 16

    nc = bacc.Bacc("TRN2", target_bir_lowering=False, debug=False,
                   num_devices=N_CORES, num_swdge_queues=4)

    def inp(name, shape, dt):
        return nc.dram_tensor(name, shape, dt, kind="ExternalInput").ap()

    xT_d = inp("xT", [IN_LOCAL, PAD_SLICE], f16)
    gfT_d = inp("gfT", [IN_GLOBAL, NWIN * P], f32)
    B_d = inp("Bh", [P, NBLK * NWIN * P], f16)
    ixe_d = inp("ixe", [P, ixe_cols], i16)
    ixz_d = inp("ixz", [P, kbh_max * P // 16], i16)
    dstT_d = inp("dstT", [P, ctot], f16)
    iota_d = inp("iota", [P, kbh_max * P], f16)
    Wg_d = inp("Wg", [IN_GLOBAL, HIDDEN], f32)
    Wc1_d = inp("Wc1", [IN_LOCAL, HIDDEN], f16)
    Wc2_d = inp("Wc2", [HIDDEN, HIDDEN], f32)
    Wm_d = inp("Wm", [HIDDEN, HIDDEN], f16)
    Ws_d = inp("Ws", [HIDDEN, HIDDEN], f16)
    Wo_d = inp("Wo", [HIDDEN, NUM_CLASSES], f16)
    bg_d = inp("bg_c", [HIDDEN, 1], f32)
    bc_d = inp("bc_b", [P, HIDDEN], f32)
    bm_d = inp("bm_b", [P, HIDDEN], f32)
    bs_d = inp("bs_c", [HIDDEN, 1], f32)
    bo_d = inp("bo_c", [NUM_CLASSES, 1], f32)
    id16_d = inp("id16", [P, P], f16)
    outT_d = nc.dram_tensor("outT", [NUM_CLASSES, SLICE], f32,
                            kind="ExternalOutput").ap()

    m_sl = [nc.dram_tensor(f"m_slice{h}", [HNODES, HIDDEN], f16).ap()
            for h in range(NHALF)]
    m_fl = [nc.dram_tensor(f"m_full{h}", [HALF_ROWS, HIDDEN], f16,
                           addr_space="Shared").ap()
            for h in range(NHALF)]

    AF = mybir.ActivationFunctionType
    OP = mybir.AluOpType

    with tile.TileContext(nc) as tc:
        with (
            tc.tile_pool(name="const", bufs=1) as cpool,
            tc.tile_pool(name="persist", bufs=1) as ppool,
            tc.tile_pool(name="work", bufs=3) as wpool,
            tc.tile_pool(name="sbig", bufs=6) as spool,
            tc.tile_pool(name="xg", bufs=2) as xgpool,
            tc.tile_pool(name="mg", bufs=MG_BUFS) as mgpool,
            tc.tile_pool(name="og", bufs=2) as ogpool,
            tc.tile_pool(name="ps_a", bufs=5, space="PSUM") as ps_a,
            tc.tile_pool(name="ps_b", bufs=2, space="PSUM") as ps_b,
            tc.tile_pool(name="ps_o", bufs=1, space="PSUM") as ps_o,
        ):
            nc.gpsimd.load_library(mlp)

            def ctile(name, ap, shape, dt):
                t = cpool.tile(shape, dt, tag=f"c_{name}")
                nc.sync.dma_start(t[:], ap[:])
                return t

            Wg_t = ctile("Wg", Wg_d, [IN_GLOBAL, HIDDEN], f32)
            Wc1_t = ctile("Wc1", Wc1_d, [IN_LOCAL, HIDDEN], f16)
            Wc2_t = ctile("Wc2", Wc2_d, [HIDDEN, HIDDEN], f32)
            Wm_t = ctile("Wm", Wm_d, [HIDDEN, HIDDEN], f16)
            Ws_t = ctile("Ws", Ws_d, [HIDDEN, HIDDEN], f16)
            Wo_t = ctile("Wo", Wo_d, [HIDDEN, NUM_CLASSES], f16)
            bg_t = ctile("bg", bg_d, [HIDDEN, 1], f32)
            bc_t = ctile("bc", bc_d, [P, HIDDEN], f32)
            bm_t = ctile("bm", bm_d, [P, HIDDEN], f32)
            bs_t = ctile("bs", bs_d, [HIDDEN, 1], f32)
            bo_t = ctile("bo", bo_d, [NUM_CLASSES, 1], f32)
            gfT_t = ctile("gfT", gfT_d, [IN_GLOBAL, NWIN * P], f32)
            id16_t = ctile("id16", id16_d, [P, P], f16)
            iota_t = ctile("iota", iota_d, [P, kbh_max * P], f16)
            ixz_t = ctile("ixz", ixz_d, [P, kbh_max * P // 16], i16)

            ident = cpool.tile([P, P], f32)
            make_identity(nc, ident[:])

            ixe_t = ppool.tile([P, ixe_cols], i16)
            nc.sync.dma_start(ixe_t[:], ixe_d[:])
            dstT_t = ppool.tile([P, ctot], f16)
            nc.sync.dma_start(dstT_t[:], dstT_d[:])
            B_t = ppool.tile([P, NBLK * NWIN * P], f16)
            nc.sync.dma_start(B_t[:], B_d[:])

            h0T_t = ppool.tile([HIDDEN, PAD_SLICE], f16)    # 3.2 MB
            m16_t = ppool.tile([P, PAD_SLICE], f16)         # 3.2 MB
            u_rot = ppool.tile([P, NWIN, HIDDEN], f16)

            # ------- warm-up gathers: fill mg pool slots with finite f16 ---
            warm_tiles = []
            for wi in range(MG_BUFS):
                mgt = mgpool.tile([P, kbh_max, HIDDEN], f16, tag="mg")
                nc.gpsimd.dma_gather(
                    mgt[:], id16_d[:],
                    ixz_t[:], kbh_max * P, kbh_max * P, HIDDEN,
                    single_packet=False, queue_num=wi % 4)
                warm_tiles.append(mgt)

            # ---------------- phase 0: global encoder (rotated) ----------
            for w in range(NWIN):
                wsl = slice(w * P, (w + 1) * P)
                ps1 = ps_b.tile([P, P], f32, tag="pb")
                nc.tensor.matmul(out=ps1[:], lhsT=Wg_t[:], rhs=gfT_t[:, wsl],
                                 start=True, stop=True)
                rT = wpool.tile([P, P], f32, tag="rT")
                nc.scalar.activation(out=rT[:], in_=ps1[:], func=AF.Relu,
                                     bias=bg_t[:, :1])
                ps2 = ps_b.tile([P, P], f32, tag="pb")
                nc.tensor.matmul(out=ps2[:], lhsT=Wc2_t[:], rhs=rT[:],
                                 start=True, stop=True)
                uc = wpool.tile([P, P], f32, tag="uc")
                nc.vector.tensor_copy(out=uc[:], in_=ps2[:])
                ps3 = ps_b.tile([P, P], f32, tag="pb")
                nc.tensor.transpose(out=ps3[:], in_=uc[:], identity=ident[:])
                nc.vector.tensor_tensor(out=u_rot[:, w, :], in0=ps3[:],
                                        in1=bc_t[:], op=OP.add)

            # ---------------- phase 1 (per half) + half AGs ----------------
            def phase1_half(h):
                for gg in range(HBLK // G):
                    b0 = h * HBLK + gg * G
                    gsl = slice(b0 * P, (b0 + G) * P)
                    xg = xgpool.tile([IN_LOCAL, G * P], f16, tag="xg")
                    nc.sync.dma_start(xg[:], xT_d[:, gsl])
                    for j in range(G):
                        b = b0 + j
                        bsl = slice(b * P, (b + 1) * P)
                        psh = ps_b.tile([P, P], f32, tag="pb")
                        nc.tensor.matmul(out=psh[:], lhsT=Wc1_t[:],
                                         rhs=xg[:, j * P:(j + 1) * P],
                                         start=True, stop=False)
                        for w in range(NWIN):
                            c0 = (b * NWIN + w) * P
                            nc.tensor.matmul(out=psh[:],
                                             lhsT=u_rot[:, w, :],
                                             rhs=B_t[:, c0:c0 + P],
                                             start=False, stop=(w == NWIN - 1))
                        nc.vector.tensor_scalar_max(out=h0T_t[:, bsl],
                                                    in0=psh[:], scalar1=0.0)
                        psm = ps_b.tile([P, P], f32, tag="pb")
                        nc.tensor.matmul(out=psm[:], lhsT=h0T_t[:, bsl],
                                         rhs=Wm_t[:], start=True, stop=True)
                        nc.vector.tensor_tensor(out=m16_t[:, bsl],
                                                in0=psm[:], in1=bm_t[:],
                                                op=OP.add)
                        nc.vector.tensor_scalar_max(out=m16_t[:, bsl],
                                                    in0=m16_t[:, bsl],
                                                    scalar1=0.0)
                    lsl = slice((b0 - h * HBLK) * P, (b0 - h * HBLK + G) * P)
                    nc.sync.dma_start(
                        m_sl[h][lsl, :].rearrange("(b p) f -> p b f", p=P),
                        m16_t[:, gsl].rearrange("p (b f) -> p b f", f=HIDDEN))

            rg = [list(range(N_CORES))]
            phase1_half(0)
            nc.gpsimd.collective_compute(
                "AllGather", OP.bypass, replica_groups=rg,
                ins=[m_sl[0][:]], outs=[m_fl[0][:]])
            phase1_half(1)

            # ---------------- phase 2: pipelined scatter-add ------------
            iota_v = iota_t[:].rearrange("p (k f) -> p k f", k=kbh_max)
            coff = np.zeros((NBLK, NHALF), np.int64)
            acc = 0
            for b in range(NBLK):
                for h in range(NHALF):
                    coff[b][h] = acc
                    acc += kbh[b][h]

            pa_open = {}
            og_box = {"t": None}
            call_i = {"n": MG_BUFS}

            def issue_half(b, h, close):
                k = kbh[b][h]
                pa = pa_open[b]
                if k:
                    c0 = int(coff[b][h])
                    n = int(nbh[b][h])
                    ci = call_i["n"]
                    call_i["n"] += 1
                    mgt = mgpool.tile([P, kbh_max, HIDDEN], f16, tag="mg")
                    nc.gpsimd.dma_gather(
                        mgt[:, :k, :], m_fl[h][IDX_BIAS:HALF_ROWS, :],
                        ixe_t[:, c0 * 8:(c0 + k) * 8], n, n, HIDDEN,
                        single_packet=False, queue_num=ci % 4)
                    S = spool.tile([P, kbh_max, P], f16, tag="S")
                    nc.vector.tensor_tensor(
                        out=S[:, :k, :],
                        in0=dstT_t[:, c0:c0 + k].to_broadcast([P, k, P]),
                        in1=iota_v[:, :k, :], op=OP.is_equal)
                    for kk in range(k):
                        nc.tensor.matmul(out=pa[:], lhsT=mgt[:, kk, :],
                                         rhs=S[:, kk, :],
                                         start=False, stop=False)
                if not close:
                    return
                bsl = slice(b * P, (b + 1) * P)
                nc.tensor.matmul(out=pa[:], lhsT=Ws_t[:], rhs=h0T_t[:, bsl],
                                 start=False, stop=True)
                del pa_open[b]
                hT = wpool.tile([HIDDEN, P], f16, tag="hT")
                nc.scalar.activation(out=hT[:], in_=pa[:], func=AF.Relu,
                                     bias=bs_t[:, :1])
                po = ps_o.tile([NUM_CLASSES, P], f32, tag="po")
                nc.tensor.matmul(out=po[:], lhsT=Wo_t[:], rhs=hT[:],
                                 start=True, stop=True)
                j = b % G
                if j == 0:
                    og_new = ogpool.tile([NUM_CLASSES, G * P], f32, tag="og")
                    og_box["t"] = og_new
                og = og_box["t"]
                nc.scalar.activation(out=og[:, j * P:(j + 1) * P], in_=po[:],
                                     func=AF.Identity, bias=bo_t[:, :1])
                if j == G - 1:
                    g0 = (b - j) * P
                    wd = min(SLICE, g0 + G * P) - g0
                    nc.sync.dma_start(outT_d[:, g0:g0 + wd], og[:, :wd])

            for t in range(NBLK + D):
                if t == D:
                    # AG1 sits after the first D half-0 gathers so they run
                    # while phase-1 half 1 finishes; its wait costs GpSimd
                    # only the collective latency itself.
                    nc.gpsimd.collective_compute(
                        "AllGather", OP.bypass, replica_groups=rg,
                        ins=[m_sl[1][:]], outs=[m_fl[1][:]])
                if t < NBLK:
                    b = t
                    bsl = slice(b * P, (b + 1) * P)
                    pa = ps_a.tile([HIDDEN, P], f32, tag="pa")
                    pa_open[b] = pa
                    nc.tensor.matmul(out=pa[:], lhsT=m16_t[:, bsl],
                                     rhs=id16_t[:], start=True, stop=False)
                    issue_half(b, 0, close=False)
                if t >= D:
                    issue_half(t - D, 1, close=True)

    nc.compile()
    return nc


# --------------------------------------------------------------------------
# host side
# --------------------------------------------------------------------------
def _wrap16(ix):
    """dma_gather int16 index layout: [16, n/16] wrapped, tiled to 128 parts."""
    return np.tile(ix.reshape(-1, 16).T, (8, 1))


def _preprocess(inputs):
    x = np.asarray(inputs["x"], dtype=np.float32)
    ei = np.asarray(inputs["edge_index"]).astype(np.int64)
    batch = np.asarray(inputs["batch"]).astype(np.int64)
    gf = np.asarray(inputs["global_feat"], dtype=np.float32)
    W = {k: np.ascontiguousarray(np.asarray(inputs[k], dtype=np.float32))
         for k in ("Wg", "bg", "Wc", "bc", "Wm", "bm", "Ws", "bs", "Wo", "bo")}

    src_all, dst_all = ei[0], ei[1]
    core_s = src_all // SLICE
    loc_s = src_all % SLICE
    half_all = loc_s // HNODES
    idx16_all = (core_s * HNODES + loc_s % HNODES - IDX_BIAS).astype(np.int16)
    core_of = dst_all // SLICE

    per_core = []
    counts = np.zeros((N_CORES, NBLK, NHALF), np.int64)
    for c in range(N_CORES):
        sel = np.nonzero(core_of == c)[0]
        d_loc = dst_all[sel] - c * SLICE
        blk = d_loc // P
        h = half_all[sel]
        key = blk * NHALF + h
        order = np.lexsort((idx16_all[sel], key))
        sel = sel[order]
        cnt = np.bincount(key[order], minlength=NBLK * NHALF
                          ).reshape(NBLK, NHALF)
        counts[c] = cnt
        per_core.append((sel, (d_loc[order] % P).astype(np.float16),
                         idx16_all[sel], cnt))

    nbh = counts.max(axis=0)                      # equalized counts [NBLK,2]
    kbh = -(-nbh // P)                            # chunk capacities
    kbh_max = int(kbh.max())
    ctot = int(kbh.sum())

    iota_np = np.tile(np.arange(P, dtype=np.float16), (P, kbh_max))
    shared = {
        "iota": iota_np,
        "id16": np.eye(P, dtype=np.float16),
        "ixz": np.zeros((P, kbh_max * P // 16), np.int16),
        "Wg": W["Wg"],
        "Wc1": np.ascontiguousarray(W["Wc"][:IN_LOCAL]).astype(np.float16),
        "Wc2": np.ascontiguousarray(W["Wc"][IN_LOCAL:]),
        "Wm": W["Wm"].astype(np.float16),
        "Ws": W["Ws"].astype(np.float16),
        "Wo": W["Wo"].astype(np.float16),
        "bg_c": W["bg"].reshape(HIDDEN, 1),
        "bc_b": np.tile(W["bc"], (P, 1)),
        "bm_b": np.tile(W["bm"], (P, 1)),
        "bs_c": W["bs"].reshape(HIDDEN, 1),
        "bo_c": W["bo"].reshape(NUM_CLASSES, 1),
    }

    in_maps = []
    for c in range(N_CORES):
        sel, d128, l16, cnt = per_core[c]
        flat_cnt = cnt.reshape(-1)
        starts = np.cumsum(flat_cnt) - flat_cnt
        dstT = np.full((ctot, P), -1.0, np.float16)
        ix_all = np.zeros(ctot * P, np.int16)
        c_acc = 0
        for b in range(NBLK):
            for h in range(NHALF):
                n = int(cnt[b][h])
                s0 = int(starts[b * NHALF + h])
                k = int(kbh[b][h])
                ix_all[c_acc * P:c_acc * P + n] = l16[s0:s0 + n]
                dd = dstT[c_acc:c_acc + k].reshape(-1)
                dd[:n] = d128[s0:s0 + n]
                c_acc += k
        ixe = _wrap16(ix_all)
        dstT_in = np.ascontiguousarray(dstT.T)

        # rotated graph space for the u expansion
        base = int(batch[c * SLICE])
        gsel = (base + np.arange(NWIN * P)) % N_GRAPHS
        gfT_rot = np.ascontiguousarray(gf[gsel].T)
        bpad = np.empty(PAD_SLICE, np.int64)
        bpad[:SLICE] = batch[c * SLICE:(c + 1) * SLICE]
        bpad[SLICE:] = bpad[SLICE - 1]
        brot = (bpad - base) % N_GRAPHS
        assert brot.max() < NWIN * P, f"core {c}: rotated graph overflow"
        Bh = np.zeros((P, NBLK * NWIN * P), np.float16)
        nidx = np.arange(PAD_SLICE)
        col = (nidx // P) * (NWIN * P) + (brot // P) * P + nidx % P
        Bh[brot % P, col] = 1.0

        xT = np.zeros((IN_LOCAL, PAD_SLICE), np.float16)
        xT[:, :SLICE] = x[c * SLICE:(c + 1) * SLICE].T.astype(np.float16)

        m = dict(shared)
        m.update({"xT": xT, "gfT": gfT_rot, "Bh": Bh, "ixe": ixe,
                  "dstT": dstT_in})
        in_maps.append(m)

    kbh_t = tuple(tuple(int(v) for v in r) for r in kbh)
    nbh_t = tuple(tuple(int(v) for v in r) for r in nbh)
    return kbh_t, nbh_t, in_maps


# --------------------------------------------------------------------------
# profiling hook (NTFF via the axon PJRT .so; absent module in this image)
# --------------------------------------------------------------------------
def _profile_hook():
    so = "/opt/axon/libaxon_pjrt.so"
    if not os.path.exists(so):
        return None
    lib = ctypes.CDLL(so)
    if not hasattr(lib, "axon_start_nrt_profile"):
        return None
    lib.axon_start_nrt_profile.argtypes = [ctypes.POINTER(ctypes.c_int64),
                                           ctypes.c_size_t]
    lib.axon_start_nrt_profile.restype = ctypes.c_int64
    lib.axon_stop_nrt_profile.argtypes = [ctypes.c_char_p]
    lib.axon_stop_nrt_profile.restype = ctypes.c_int64

    @contextlib.contextmanager
    def hook(output_dir, device_ids):
        import jax
        jax.devices()
        if device_ids:
            ids = (ctypes.c_int64 * len(device_ids))(*device_ids)
            rc = lib.axon_start_nrt_profile(ids, len(device_ids))
        else:
            rc = lib.axon_start_nrt_profile(None, 0)
        if rc != 0:
            raise RuntimeError(f"axon_start_nrt_profile rc={rc}")
        try:
            yield
        finally:
            n = lib.axon_stop_nrt_profile(str(output_dir).encode())
            print(f"profile: {n} file(s) written to {output_dir}",
                  file=sys.stderr)

    return hook


def _run(nc, in_maps):
    from concourse import bass2jax
    trace_dir = os.environ.get("GNN_TRACE_DIR", "")
    if not trace_dir:
        return bass2jax.run_bass_via_pjrt(nc, in_maps, n_cores=N_CORES)
    hook = _profile_hook()
    if hook is None:
        return bass2jax.run_bass_via_pjrt(nc, in_maps, n_cores=N_CORES)
    import time as _time
    trace_dir = os.path.join(trace_dir, f"run_{int(_time.time()*1000)}")
    os.makedirs(trace_dir, exist_ok=True)
    last_run["trace_dir"] = trace_dir
    trace_cores = [int(t) for t in
                   os.environ.get("GNN_TRACE_CORES", "0").split(",")]
    with hook(trace_dir, trace_cores):
        results = bass2jax.run_bass_via_pjrt(nc, in_maps, n_cores=N_CORES)
    try:
        from concourse._compat import FishPath
        import gauge.profiler as gprof
        profile = gprof.Profile(
            profile_path=FishPath(trace_dir), kernel_dev_mode=True,
            profile_on_exit=False, bass_kernel=nc.m,
            offline_processing=True, fname="*_body*")
        profile.convert_ntffs_to_json(tuple(trace_cores))
        j = profile.load_json(trace_cores[0])
        last_run["summary"] = j["summary"][0] if j else None
        last_run["exec_time_ns"] = (
            int(j["summary"][0]["total_time"] * 1e9) if j else None)
        last_run["profile_json"] = str(profile.json_path(trace_cores[0]))
    except Exception as e:  # profiling must never break the run
        print(f"profile post-processing failed: {e}", file=sys.stderr)
    return results


def kernel(**inputs) -> np.ndarray:
    kbh, nbh, in_maps = _preprocess(inputs)
    key = (kbh, nbh)
    nc = _prog_cache.get(key)
    if nc is None:
        nc = _build(kbh, nbh)
        _prog_cache[key] = nc
    last_run.clear()
    results = _run(nc, in_maps)
    outT = np.concatenate([results[c]["outT"] for c in range(N_CORES)], axis=1)
    return np.ascontiguousarray(outT.T.astype(np.float32))
